# revision 3
# baseline (speedup 1.0000x reference)
"""Trainium2 Bass kernel for nn_SNSCell (gnn_message_passing).

Math (per batch row b, feature j, n=128):
    Gm,bm,Gmax,Esyn are clipped; ge[j] = sum_i Gmax[i,j]*Esyn[i,j]
    P = h @ Gmax
    out[b,j] = (1-Gm[j])*h[b,j] + bm[j] + i_app[b,j]
               + clamp01(h[b,j]) * (ge[j] - P[b,j])

Strategy: data-parallel over batch across 8 cores (32768 rows each).
All HBM I/O in bf16 (host casts inputs; host upcasts the bf16 output),
halving memory traffic vs fp32. On-chip:
  - PE-transpose bf16 h blocks [128b,128i] -> hT [128i,128b]
  - Act copies hT to SBUF; PSUM Q = -P^T via negG matmul
  - Act: d = ge - P^T (PSUM src, per-partition bias)
  - DVE: cl = clamp01(hT); t = cl * d
  - PE folds the rest: OT[b,m] = sum_j ht[j,b]*diagOmg[j,m] (= (1-Gm)*h)
    accumulated with sum_j t[j,b]*I[j,m] (= t^T), so the (1-Gm)*h term
    and the back-transpose cost no extra DVE passes.
  - DVE: oc = OT + i_app (bf16, bm folded in on host), store bf16.
"""

import numpy as np
import ml_dtypes
from contextlib import ExitStack

import concourse.bacc as bacc
import concourse.tile as tile
from concourse import mybir
from concourse.bass_utils import run_bass_kernel_spmd

B_FULL = 262144
N = 128
N_CORES = 8
ROWS = B_FULL // N_CORES          # 32768 rows per core
RPP = 8                           # rows packed per partition per DMA chunk
CHUNK_COLS = RPP * N              # 2048 cols per chunk tile
N_CHUNKS = ROWS // (128 * RPP)    # 16 chunks of [128, 2048] (512 KiB bf16)
SUPER = 1024                      # cols per compute super-tile
N_SUPER = CHUNK_COLS // SUPER     # 2 super-tiles per chunk

F32 = mybir.dt.float32
BF16 = mybir.dt.bfloat16
AOT = mybir.AluOpType
ACT_F = mybir.ActivationFunctionType
BF = ml_dtypes.bfloat16

_CACHE = {}


def _build():
    nc = bacc.Bacc("TRN2", debug=False)

    h = nc.dram_tensor("h", [ROWS, N], BF16, kind="ExternalInput").ap()
    ia = nc.dram_tensor("ia", [ROWS, N], BF16, kind="ExternalInput").ap()
    negG = nc.dram_tensor("negG", [N, N], BF16, kind="ExternalInput").ap()
    identb = nc.dram_tensor("identb", [N, N], BF16, kind="ExternalInput").ap()
    diagOmg = nc.dram_tensor("diagOmg", [N, N], BF16, kind="ExternalInput").ap()
    ge = nc.dram_tensor("ge", [N, 1], F32, kind="ExternalInput").ap()
    out = nc.dram_tensor("out", [ROWS, N], BF16, kind="ExternalOutput").ap()

    hv = h.rearrange("(n p r) m -> n p (r m)", p=128, r=RPP)
    iav = ia.rearrange("(n p r) m -> n p (r m)", p=128, r=RPP)
    outv = out.rearrange("(n p r) m -> n p (r m)", p=128, r=RPP)

    with tile.TileContext(nc) as tc:
        with ExitStack() as ctx:
            const = ctx.enter_context(tc.tile_pool(name="const", bufs=1))
            io = ctx.enter_context(tc.tile_pool(name="io", bufs=4))
            mid = ctx.enter_context(tc.tile_pool(name="mid", bufs=3))
            pst = ctx.enter_context(tc.tile_pool(name="pst", bufs=2, space="PSUM"))
            psq = ctx.enter_context(tc.tile_pool(name="psq", bufs=2, space="PSUM"))
            pso = ctx.enter_context(tc.tile_pool(name="pso", bufs=2, space="PSUM"))

            negG_s = const.tile([N, N], BF16, tag="negG")
            ident_s = const.tile([N, N], BF16, tag="ident")
            diagOmg_s = const.tile([N, N], BF16, tag="diagOmg")
            ge_s = const.tile([N, 1], F32, tag="ge")
            nc.sync.dma_start(negG_s[:], negG[:])
            nc.sync.dma_start(ident_s[:], identb[:])
            nc.sync.dma_start(diagOmg_s[:], diagOmg[:])
            nc.sync.dma_start(ge_s[:], ge[:])

            for n in range(N_CHUNKS):
                hb = io.tile([128, CHUNK_COLS], BF16, tag="hb")
                iac = io.tile([128, CHUNK_COLS], BF16, tag="iac")
                oc = io.tile([128, CHUNK_COLS], BF16, tag="oc")
                nc.sync.dma_start(hb[:], hv[n])
                nc.sync.dma_start(iac[:], iav[n])

                for s in range(N_SUPER):
                    sl = slice(s * SUPER, (s + 1) * SUPER)

                    # hT blocks: [128 i, SUPER b], bf16
                    T = pst.tile([128, SUPER], BF16, tag="T")
                    for r in range(SUPER // 128):
                        c0 = s * SUPER + r * 128
                        nc.tensor.transpose(
                            T[:, r * 128 : (r + 1) * 128],
                            hb[:, c0 : c0 + 128],
                            ident_s[:],
                        )
                    ht = mid.tile([128, SUPER], BF16, tag="ht")
                    nc.scalar.copy(ht[:], T[:])

                    # Q = -P^T  (two N=512 matmuls, one per PSUM bank)
                    Q = psq.tile([128, SUPER], F32, tag="Q")
                    nc.tensor.matmul(
                        Q[:, 0:512], negG_s[:], ht[:, 0:512], start=True, stop=True
                    )
                    nc.tensor.matmul(
                        Q[:, 512:1024], negG_s[:], ht[:, 512:1024], start=True, stop=True
                    )

                    # d = ge - P^T   (ACT, PSUM src, per-partition bias)
                    d = mid.tile([128, SUPER], BF16, tag="d")
                    nc.scalar.activation(
                        d[:], Q[:], ACT_F.Identity, bias=ge_s[:], scale=1.0
                    )
                    # cl = clamp01(hT)
                    cl = mid.tile([128, SUPER], BF16, tag="cl")
                    nc.vector.tensor_scalar(
                        cl[:], ht[:], 0.0, 1.0, AOT.max, AOT.min
                    )
                    # t = cl * (ge - P^T)
                    t = mid.tile([128, SUPER], BF16, tag="t")
                    nc.vector.tensor_mul(t[:], cl[:], d[:])

                    # OT[b,m] = sum_j ht[j,b]*diagOmg[j,m] + sum_j t[j,b]*I[j,m]
                    #         = (1-Gm[m])*h[b,m] + t^T[b,m]     (natural layout)
                    # fp32 PSUM required for non-transpose matmul; 512-col
                    # tiles keep PSUM within 8 banks.
                    for q in range(SUPER // 512):
                        OT = pso.tile([128, 512], F32, tag="OT")
                        for r in range(4):
                            rs = slice(q * 512 + r * 128, q * 512 + (r + 1) * 128)
                            os_ = slice(r * 128, (r + 1) * 128)
                            nc.tensor.matmul(
                                OT[:, os_], ht[:, rs], diagOmg_s[:],
                                start=True, stop=False,
                            )
                            nc.tensor.matmul(
                                OT[:, os_], t[:, rs], ident_s[:],
                                start=False, stop=True,
                            )
                        # out = OT + i_app (+bm)
                        osl = slice(s * SUPER + q * 512, s * SUPER + (q + 1) * 512)
                        nc.vector.tensor_add(oc[:, osl], OT[:], iac[:, osl])

                nc.sync.dma_start(outv[n], oc[:])

    nc.compile()
    return nc


def _get_nc():
    if "nc" not in _CACHE:
        _CACHE["nc"] = _build()
    return _CACHE["nc"]


def make_in_maps(i_app, hidden, Gm, bm, Gmax, Esyn):
    i_app = np.asarray(i_app, dtype=np.float32)
    hidden = np.asarray(hidden, dtype=np.float32)
    Gm_c = np.clip(np.asarray(Gm, np.float32), 0.01, 1.0)
    bm_c = np.clip(np.asarray(bm, np.float32), -1.0, 1.0)
    Gmax_c = np.clip(np.asarray(Gmax, np.float32), 0.0, 1.0)
    Esyn_c = np.clip(np.asarray(Esyn, np.float32), -3.0, 3.0)

    ge = np.sum(Gmax_c * Esyn_c, axis=0, dtype=np.float32)  # [N]

    params = {
        "negG": np.ascontiguousarray((-Gmax_c).astype(BF)),
        "identb": np.eye(N, dtype=BF),
        "diagOmg": np.ascontiguousarray(np.diag(1.0 - Gm_c).astype(BF)),
        "ge": np.ascontiguousarray(ge.reshape(N, 1)),
    }
    h_b = hidden.astype(BF)
    ia_b = (i_app + bm_c[None, :]).astype(BF)  # fold bm into i_app
    in_maps = []
    for k in range(N_CORES):
        rows = slice(k * ROWS, (k + 1) * ROWS)
        in_maps.append(
            {
                "h": np.ascontiguousarray(h_b[rows]),
                "ia": np.ascontiguousarray(ia_b[rows]),
                **params,
            }
        )
    return in_maps


def kernel(i_app, hidden, Gm, bm, Gmax, Esyn):
    nc = _get_nc()
    in_maps = make_in_maps(i_app, hidden, Gm, bm, Gmax, Esyn)
    res = run_bass_kernel_spmd(nc, in_maps, core_ids=list(range(N_CORES)))
    out = np.concatenate(
        [res.results[k]["out"].astype(np.float32) for k in range(N_CORES)], axis=0
    )
    return (out, out)
